# revision 1
# baseline (speedup 1.0000x reference)
"""Trainium2 Bass kernel for nn_AttnBlock (GroupNorm + 4-head attention + output proj).

Sharding: 8 cores = (batch b in {0,1}) x (head h in {0..3}).  Each core computes
the full attention for its (b, h) pair plus the partial output projection
wo[:, head_cols] @ att_out_head -> [512, 4096].  The host sums the 4 head
partials per batch and adds the residual x and output bias bo (gather/unshard).

Per-core kernel (fp32 data, float32r matmuls — 4x the fp32 PE rate):
  1. GroupNorm(32 groups): bn_stats per 128-channel chunk, group aggregation via
     PE transposes of the per-channel stats, applied as h = x*A + B on ACT.
     Stats/apply run per chunk-pair so projections start before stats finish.
  2. q = wq_h h, k = wk_h h  ([128, 4096], channels on partitions),
     v likewise then transposed on the PE into vT [4096, 128] (32 tiles).
  3. Per 512-query group g (S^T layout -- no transposes in the hot loop):
       S^T[j,i] = k^T q        32 matmuls [128j, 512i], chunk-pairs in PSUM
       P = exp(scale * S^T)    one ACT exp per pair, rounded to f32r
       den = ones^T P          32 ones-matrix matmuls -> [128, 512] (all rows equal)
       out^T = V P             32 accumulating matmuls -> [128c, 512i]
       ot = out^T * (1/den)    DVE reciprocal + multiply
       yp[oc] = wo_h[oc]^T ot  4 matmuls, scaled copies DMA'd out
"""

import sys

sys.path.insert(0, "/opt/trn_rl_repo")

import numpy as np

C = 512
HEADS = 4
HC = 128          # head channels
N = 4096          # h*w pixels
P = 128           # partitions
NCH = C // P      # 4 channel chunks
NJT = N // P      # 32 key tiles
IG = 512          # query-group width
NIG = N // IG     # 8 query groups
GSIZE = 16        # channels per groupnorm group
EPS = 1e-6
SCALE = float(C) ** -0.5

_NC_CACHE = {}


def _build_nc():
    from contextlib import ExitStack

    import concourse.bacc as bacc
    import concourse.bass as bass
    import concourse.tile as tile
    from concourse import mybir
    from concourse.masks import make_identity

    f32 = mybir.dt.float32
    f32r = mybir.dt.float32r

    AF = mybir.ActivationFunctionType
    OP = mybir.AluOpType
    AX = mybir.AxisListType

    nc = bacc.Bacc("TRN2", target_bir_lowering=False, debug=False)

    xb = nc.dram_tensor("xb", [C, N], f32r, kind="ExternalInput").ap()
    wqt = nc.dram_tensor("wqt", [C, HC], f32r, kind="ExternalInput").ap()
    wkt = nc.dram_tensor("wkt", [C, HC], f32r, kind="ExternalInput").ap()
    wvt = nc.dram_tensor("wvt", [C, HC], f32r, kind="ExternalInput").ap()
    wot = nc.dram_tensor("wot", [HC, C], f32r, kind="ExternalInput").ap()
    bqh = nc.dram_tensor("bqh", [HC, 1], f32, kind="ExternalInput").ap()
    bkh = nc.dram_tensor("bkh", [HC, 1], f32, kind="ExternalInput").ap()
    bvh = nc.dram_tensor("bvh", [HC, 1], f32, kind="ExternalInput").ap()
    gns = nc.dram_tensor("gns", [1, C], f32, kind="ExternalInput").ap()
    gnb = nc.dram_tensor("gnb", [1, C], f32, kind="ExternalInput").ap()
    yp = nc.dram_tensor("yp", [C, N], f32, kind="ExternalOutput").ap()

    xbv = xb.rearrange("(a p) n -> a p n", p=P)            # [4, 128, 4096]
    wqv = wqt.rearrange("(a p) o -> p a o", p=P)           # [128, 4, 128]
    wkv = wkt.rearrange("(a p) o -> p a o", p=P)
    wvv = wvt.rearrange("(a p) o -> p a o", p=P)
    ypv = yp.rearrange("(oc p) (g i) -> oc p g i", p=P, i=IG)  # [4, 128, 8, 512]

    with tile.TileContext(nc) as tc, ExitStack() as ctx:
        consts = ctx.enter_context(tc.tile_pool(name="consts", bufs=1))
        qkv = ctx.enter_context(tc.tile_pool(name="qkv", bufs=1))
        otp = ctx.enter_context(tc.tile_pool(name="otp", bufs=2))
        yfp = ctx.enter_context(tc.tile_pool(name="yfp", bufs=2))
        bcp = ctx.enter_context(tc.tile_pool(name="bcp", bufs=2))
        pps = ctx.enter_context(tc.tile_pool(name="pps", bufs=2, space="PSUM"))

        # prologue-scoped pools (space reclaimed before the attention pools open)
        pro = ExitStack()
        xpool = pro.enter_context(tc.tile_pool(name="xpool", bufs=1))
        stats = pro.enter_context(tc.tile_pool(name="stats", bufs=1))
        stats2 = pro.enter_context(tc.tile_pool(name="stats2", bufs=2))
        ppt = pro.enter_context(tc.tile_pool(name="ppt", bufs=2, space="PSUM"))
        ppsm = pro.enter_context(tc.tile_pool(name="ppsm", bufs=2, space="PSUM"))

        # ---- constants / weights ----
        ident = consts.tile([P, P], f32)
        make_identity(nc, ident)
        ones_mat = consts.tile([P, P], f32)
        nc.vector.memset(ones_mat, 1.0)
        ones_r = consts.tile([P, P], f32r)
        nc.vector.tensor_copy(out=ones_r, in_=ones_mat)
        eps4 = consts.tile([NCH, 1], f32)
        nc.vector.memset(eps4, EPS)
        zero1 = consts.tile([P, 1], f32)
        nc.vector.memset(zero1, 0.0)
        # GN-folded projection weights: wX_s[:, c, :] = wX[:, c, :] * A_c
        wq_s = consts.tile([P, NCH, HC], f32r)
        wk_s = consts.tile([P, NCH, HC], f32r)
        wv_s = consts.tile([P, NCH, HC], f32r)

        # ---- load x first (critical path), 8 slices per channel chunk so a
        # chunk's statistics can start as soon as that chunk's queues drain ----
        xcs = [xpool.tile([P, N], f32r, name=f"xch{i}", tag=f"xch{i}") for i in range(NCH)]
        NSL = N // 8
        for ci in range(NCH):
            for sl in range(8):
                nc.sync.dma_start(
                    out=xcs[ci][:, sl * NSL : (sl + 1) * NSL],
                    in_=xbv[ci][:, sl * NSL : (sl + 1) * NSL],
                )

        w_q = consts.tile([P, NCH, HC], f32r)
        nc.sync.dma_start(out=w_q, in_=wqv)
        w_k = consts.tile([P, NCH, HC], f32r)
        nc.sync.dma_start(out=w_k, in_=wkv)
        w_v = consts.tile([P, NCH, HC], f32r)
        nc.sync.dma_start(out=w_v, in_=wvv)
        w_o = consts.tile([P, C], f32r)
        nc.sync.dma_start(out=w_o, in_=wot)
        bq_sb = consts.tile([P, 1], f32)
        nc.sync.dma_start(out=bq_sb, in_=bqh)
        bk_sb = consts.tile([P, 1], f32)
        nc.sync.dma_start(out=bk_sb, in_=bkh)
        bv_sb = consts.tile([P, 1], f32)
        nc.sync.dma_start(out=bv_sb, in_=bvh)
        # gn scale/bias as two [2, 128] tiles (base partition 0) per chunk-pair
        gns_h = [consts.tile([2, P], f32, name=f"gns{h}", tag=f"gns{h}") for h in range(2)]
        gnb_h = [consts.tile([2, P], f32, name=f"gnb{h}", tag=f"gnb{h}") for h in range(2)]
        gnsv = gns.rearrange("a (b c) -> (a b) c", b=NCH)
        gnbv = gnb.rearrange("a (b c) -> (a b) c", b=NCH)
        for h in range(2):
            nc.sync.dma_start(out=gns_h[h], in_=gnsv[2 * h : 2 * h + 2, :])
            nc.sync.dma_start(out=gnb_h[h], in_=gnbv[2 * h : 2 * h + 2, :])

        # ---- GroupNorm ----
        # Every 16-channel group lives inside one 128-channel chunk, so the
        # stats -> apply chain runs per chunk-PAIR: the projections over
        # chunks 0/1 start while chunks 2/3 are still in bn_stats.
        mv = stats.tile([P, NCH, 2], f32)
        acol = stats.tile([P, NCH], f32)
        bcol = stats.tile([P, NCH], f32r)

        def gn_half(h):
            lo = 2 * h
            # per-channel mean/var for the two chunks
            for ci in (lo, lo + 1):
                st = stats2.tile([P, 8, 6], f32, name="st", tag="st")
                xv = xcs[ci][:].bitcast(f32).rearrange("p (s f) -> p s f", f=512)
                for s in range(8):
                    nc.vector.bn_stats(out=st[:, s, :], in_=xv[:, s, :])
                nc.vector.bn_aggr(out=mv[:, ci, :], in_=st)
            # vpm = var + mean^2
            vpm = stats.tile([P, 2], f32, name=f"vpm{h}", tag=f"vpm{h}")
            nc.vector.tensor_mul(vpm, mv[:, lo : lo + 2, 0], mv[:, lo : lo + 2, 0])
            nc.vector.tensor_add(vpm, vpm, mv[:, lo : lo + 2, 1])
            # transpose to chunk-major rows [2, 128]
            mrow = stats.tile([2, P], f32, name=f"mrow{h}", tag=f"mrow{h}")
            vrow = stats.tile([2, P], f32, name=f"vrow{h}", tag=f"vrow{h}")
            pmz = ppsm.tile([2, P], f32, name="pmz", tag="sm")
            nc.tensor.transpose(pmz, mv[:, lo : lo + 2, 0], ident)
            nc.vector.tensor_copy(out=mrow, in_=pmz)
            pvz = ppsm.tile([2, P], f32, name="pvz", tag="sm")
            nc.tensor.transpose(pvz, vpm, ident)
            nc.vector.tensor_copy(out=vrow, in_=pvz)
            # group means -> [2, 8]
            gm = stats.tile([2, 8], f32, name=f"gm{h}", tag=f"gm{h}")
            gv = stats.tile([2, 8], f32, name=f"gv{h}", tag=f"gv{h}")
            nc.vector.reduce_sum(
                out=gm[:], in_=mrow[:].rearrange("p (g s) -> p g s", s=GSIZE), axis=AX.X
            )
            nc.vector.tensor_scalar_mul(gm, gm, 1.0 / GSIZE)
            nc.vector.reduce_sum(
                out=gv[:], in_=vrow[:].rearrange("p (g s) -> p g s", s=GSIZE), axis=AX.X
            )
            nc.vector.tensor_scalar_mul(gv, gv, 1.0 / GSIZE)
            gmsq = stats.tile([2, 8], f32, name=f"gmsq{h}", tag=f"gmsq{h}")
            nc.vector.tensor_mul(gmsq, gm, gm)
            nc.vector.tensor_sub(gv, gv, gmsq)     # group variance
            nc.scalar.activation(out=gv, in_=gv, func=AF.Sqrt, bias=eps4[0:2, :])
            nc.vector.reciprocal(gv, gv)           # rstd per group
            # expand groups to channels [2, 128]
            grx = stats.tile([2, P], f32, name=f"grx{h}", tag=f"grx{h}")
            gmx = stats.tile([2, P], f32, name=f"gmx{h}", tag=f"gmx{h}")
            gv_ap = gv[:]
            gm_ap = gm[:]
            gv_b = bass.AP(tensor=gv_ap.tensor, offset=gv_ap.offset, ap=list(gv_ap.ap) + [[0, GSIZE]])
            gm_b = bass.AP(tensor=gm_ap.tensor, offset=gm_ap.offset, ap=list(gm_ap.ap) + [[0, GSIZE]])
            nc.vector.tensor_copy(out=grx[:].rearrange("p (g s) -> p g s", s=GSIZE), in_=gv_b)
            nc.vector.tensor_copy(out=gmx[:].rearrange("p (g s) -> p g s", s=GSIZE), in_=gm_b)
            nc.vector.tensor_mul(grx, grx, gns_h[h])
            nc.vector.tensor_mul(gmx, gmx, grx)
            nc.vector.tensor_sub(gmx, gnb_h[h], gmx)
            # back to per-partition scalars [128, 2]
            paz = ppsm.tile([P, 2], f32, name="paz", tag="sm")
            nc.tensor.transpose(paz, grx, ident[0:2, 0:2])
            nc.vector.tensor_copy(out=acol[:, lo : lo + 2], in_=paz)
            pbz = ppsm.tile([P, 2], f32, name="pbz", tag="sm")
            nc.tensor.transpose(pbz, gmx, ident[0:2, 0:2])
            nc.vector.tensor_copy(out=bcol[:, lo : lo + 2], in_=pbz)
            # fold GN into the projection weights instead of rewriting x:
            # wX_s[:, ci, :] = wX[:, ci, :] * A_ci   (tiny ACT ops; x stays raw)
            for ci in (lo, lo + 1):
                for wsrc, wdst in ((w_q, wq_s), (w_k, wk_s), (w_v, wv_s)):
                    nc.scalar.activation(
                        out=wdst[:, ci, :],
                        in_=wsrc[:, ci, :].bitcast(f32),
                        func=AF.Identity,
                        bias=zero1,
                        scale=acol[:, ci : ci + 1],
                    )

        gn_half(0)
        gn_half(1)

        # ---- projections q, k, v ----
        q_sb = qkv.tile([P, N], f32r)
        k_sb = qkv.tile([P, N], f32r)
        v_sb = xpool.tile([P, N], f32)
        vt_sb = qkv.tile([P, NJT, HC], f32r)

        for w_raw, w_sb, b_sb, dst in (
            (w_q, wq_s, bq_sb, q_sb),
            (w_k, wk_s, bk_sb, k_sb),
            (w_v, wv_s, bv_sb, v_sb),
        ):
            # bias fold: bvec = W^T B  (per output channel), added to the conv bias
            pbv = ppsm.tile([P, 1], f32, name="pbv", tag="sm")
            for ci in range(NCH):
                nc.tensor.matmul(
                    pbv,
                    lhsT=w_raw[:, ci, :].bitcast(f32),
                    rhs=bcol[:, ci : ci + 1].bitcast(f32),
                    start=(ci == 0),
                    stop=(ci == NCH - 1),
                )
            b2 = stats.tile([P, 1], f32, name="b2", tag="b2", bufs=3)
            nc.vector.tensor_add(b2, b_sb, pbv)
            for nh in range(NIG):
                pp = pps.tile([P, IG], f32, tag="ps")
                for ci in range(NCH):
                    nc.tensor.matmul(
                        pp,
                        lhsT=w_sb[:, ci, :],
                        rhs=xcs[ci][:, nh * IG : (nh + 1) * IG],
                        start=(ci == 0),
                        stop=(ci == NCH - 1),
                    )
                nc.scalar.activation(
                    out=dst[:, nh * IG : (nh + 1) * IG],
                    in_=pp,
                    func=AF.Identity,
                    bias=b2,
                    scale=1.0,
                )

        for jt in range(NJT):
            ptr = ppt.tile([P, P], f32)
            nc.tensor.transpose(ptr, v_sb[:, jt * P : (jt + 1) * P], ident)
            nc.vector.tensor_copy(out=vt_sb[:, jt, :], in_=ptr)

        pro.close()

        # attention-phase pools
        ptp = ctx.enter_context(tc.tile_pool(name="ptp", bufs=1))
        ppden = ctx.enter_context(tc.tile_pool(name="ppden", bufs=1, space="PSUM"))
        ppo = ctx.enter_context(tc.tile_pool(name="ppo", bufs=1, space="PSUM"))
        ppf = ctx.enter_context(tc.tile_pool(name="ppf", bufs=2, space="PSUM"))

        # ---- attention ----
        pt_big = ptp.tile([P, NJT, IG], f32r)
        for g in range(NIG):
            qs = q_sb[:, g * IG : (g + 1) * IG]

            # S^T chunk-pair matmuls + one exp per 1024 columns, then a
            # pair-sum on DVE/GpSimd so the denominator matmul only needs
            # 16 chunks
            for jp in range(NJT // 2):
                ps = pps.tile([P, 2, IG], f32, tag="ps")
                for h in range(2):
                    jt = 2 * jp + h
                    nc.tensor.matmul(
                        ps[:, h, :],
                        lhsT=k_sb[:, jt * P : (jt + 1) * P],
                        rhs=qs,
                        start=True,
                        stop=True,
                    )
                nc.scalar.activation(
                    out=pt_big[:, 2 * jp : 2 * jp + 2, :],
                    in_=ps,
                    func=AF.Exp,
                    scale=SCALE,
                )

            # denominators: ones-matrix matmul -> every partition holds the sums
            pden = ppden.tile([P, IG], f32)
            for jt in range(NJT):
                nc.tensor.matmul(
                    pden,
                    lhsT=ones_r,
                    rhs=pt_big[:, jt, :],
                    start=(jt == 0),
                    stop=(jt == NJT - 1),
                )

            po = ppo.tile([P, IG], f32)
            for jt in range(NJT):
                nc.tensor.matmul(
                    po,
                    lhsT=vt_sb[:, jt, :],
                    rhs=pt_big[:, jt, :],
                    start=(jt == 0),
                    stop=(jt == NJT - 1),
                )

            bc = bcp.tile([P, IG], f32)
            nc.vector.reciprocal(bc, pden)
            ot = otp.tile([P, IG], f32r)
            nc.vector.tensor_mul(ot, po, bc)

            for oc in range(NCH):
                pf = ppf.tile([P, IG], f32)
                nc.tensor.matmul(pf, lhsT=w_o[:, oc * P : (oc + 1) * P], rhs=ot, start=True, stop=True)
                yf = yfp.tile([P, IG], f32)
                nc.vector.tensor_copy(out=yf, in_=pf)
                nc.sync.dma_start(out=ypv[oc, :, g, :], in_=yf)

    nc.compile()
    return nc


def get_nc():
    if "nc" not in _NC_CACHE:
        _NC_CACHE["nc"] = _build_nc()
    return _NC_CACHE["nc"]


def make_in_maps(inputs):
    x = np.ascontiguousarray(np.asarray(inputs["x"], dtype=np.float32))
    wq = np.asarray(inputs["wq"], np.float32)
    wk = np.asarray(inputs["wk"], np.float32)
    wv = np.asarray(inputs["wv"], np.float32)
    bq = np.asarray(inputs["bq"], np.float32)
    bk = np.asarray(inputs["bk"], np.float32)
    bv = np.asarray(inputs["bv"], np.float32)
    wo = np.asarray(inputs["wo"], np.float32)
    gn_scale = np.asarray(inputs["gn_scale"], np.float32)
    gn_bias = np.asarray(inputs["gn_bias"], np.float32)

    in_maps = []
    for cid in range(8):
        b, h = divmod(cid, HEADS)
        sl = slice(h * HC, (h + 1) * HC)
        in_maps.append(
            {
                "xb": x[b].reshape(C, N),
                "wqt": np.ascontiguousarray(wq[sl, :].T),
                "wkt": np.ascontiguousarray(wk[sl, :].T),
                "wvt": np.ascontiguousarray(wv[sl, :].T),
                "wot": np.ascontiguousarray(wo[:, sl].T),
                "bqh": np.ascontiguousarray(bq[sl].reshape(HC, 1)),
                "bkh": np.ascontiguousarray(bk[sl].reshape(HC, 1)),
                "bvh": np.ascontiguousarray(bv[sl].reshape(HC, 1)),
                "gns": np.ascontiguousarray(gn_scale.reshape(1, C)),
                "gnb": np.ascontiguousarray(gn_bias.reshape(1, C)),
            }
        )
    return in_maps


def assemble_output(inputs, yps):
    x = np.asarray(inputs["x"], np.float32)
    bo = np.asarray(inputs["bo"], np.float32)
    y = x.reshape(2, C, N).astype(np.float32).copy()
    y += bo.reshape(1, C, 1)
    for cid in range(8):
        b = cid // HEADS
        y[b] += yps[cid]
    return y.reshape(2, C, 64, 64)


def run(inputs, trace=False):
    from concourse.bass_utils import run_bass_kernel_spmd

    nc = get_nc()
    in_maps = make_in_maps(inputs)
    res = run_bass_kernel_spmd(nc, in_maps, list(range(8)), trace=trace)
    yps = [r["yp"] for r in res.results]
    return assemble_output(inputs, yps), res


def kernel(**inputs):
    y, _ = run(inputs, trace=False)
    return y



# revision 4
# speedup vs baseline: 1.0872x; 1.0872x over previous
"""Trainium2 Bass kernel for nn_AttnBlock (GroupNorm + 4-head attention + output proj).

Sharding: 8 cores = (batch b in {0,1}) x (head h in {0..3}).  Each core computes
the full attention for its (b, h) pair plus the partial output projection
wo[:, head_cols] @ att_out_head -> [512, 4096] (bf16).  The host sums the 4
head partials per batch and adds the residual x and output bias bo.

v2: fp8 (float8e4) + DoubleRow perf-mode matmuls on the whole attention path.
  - x arrives as fp8 [128p, 4chunk, 4096]; GroupNorm stats (bn_stats) read the
    fp8 tile directly and are folded into the projection weights (wX_s fp8)
    and biases, so x is never rewritten.
  - q,k projections: DoubleRow over input-chunk pairs, emitted into a split
    [64, 2, 512] PSUM layout so S^T can also run DoubleRow (contraction 2x64).
    k's bias is dropped entirely: a per-query-constant shift of the scores
    cancels in softmax.  q's bias fold is computed directly in [64,2] layout.
  - S^T[j,i] DoubleRow matmuls -> exp on ACT (fp8 out, double-buffered pt8)
    -> denominator via all-ones DoubleRow matmuls -> out^T = V P DoubleRow.
  - ACT does nothing but exp in steady state (the wall: ~142us of exp).
  - DVE: bn_stats, psum->sbuf conversions, reciprocal_approx_fast, ot=po/den.
  - y written as bf16 (host upcasts and adds residual + bo).
"""

import sys

sys.path.insert(0, "/opt/trn_rl_repo")

import numpy as np
import ml_dtypes

C = 512
HEADS = 4
HC = 128          # head channels
N = 4096          # h*w pixels
P = 128           # partitions
NCH = C // P      # 4 channel chunks
NJT = N // P      # 32 key tiles
IG = 512          # query-group width
NIG = N // IG     # 8 query groups
GSIZE = 16        # channels per groupnorm group
EPS = 1e-6
SCALE = float(C) ** -0.5

_NC_CACHE = {}


def _build_nc():
    from contextlib import ExitStack

    import concourse.bacc as bacc
    import concourse.bass as bass
    import concourse.tile as tile
    from concourse import mybir
    from concourse.masks import make_identity

    f32 = mybir.dt.float32
    f32r = mybir.dt.float32r
    bf16 = mybir.dt.bfloat16
    f8 = mybir.dt.float8e4

    AF = mybir.ActivationFunctionType
    AX = mybir.AxisListType
    DR = mybir.MatmulPerfMode.DoubleRow

    nc = bacc.Bacc("TRN2", target_bir_lowering=False, debug=False)

    x8d = nc.dram_tensor("x8d", [P, NCH, N], f8, kind="ExternalInput").ap()
    wqt = nc.dram_tensor("wqt", [P, NCH, HC], f32, kind="ExternalInput").ap()
    wkt = nc.dram_tensor("wkt", [P, NCH, HC], f32, kind="ExternalInput").ap()
    wvt = nc.dram_tensor("wvt", [P, NCH, HC], f32, kind="ExternalInput").ap()
    wot = nc.dram_tensor("wot", [HC, C], f32r, kind="ExternalInput").ap()
    bq2h = nc.dram_tensor("bq2h", [64, 2], f32, kind="ExternalInput").ap()
    bvh = nc.dram_tensor("bvh", [HC, 1], f32, kind="ExternalInput").ap()
    gns = nc.dram_tensor("gns", [1, C], f32, kind="ExternalInput").ap()
    gnb = nc.dram_tensor("gnb", [1, C], f32, kind="ExternalInput").ap()
    yp = nc.dram_tensor("yp", [C, N], bf16, kind="ExternalOutput").ap()

    ypv = yp.rearrange("(oc p) (g i) -> oc p g i", p=P, i=IG)  # [4, 128, 8, 512]

    with tile.TileContext(nc) as tc, ExitStack() as ctx:
        consts = ctx.enter_context(tc.tile_pool(name="consts", bufs=1))
        qkv = ctx.enter_context(tc.tile_pool(name="qkv", bufs=1))
        otp = ctx.enter_context(tc.tile_pool(name="otp", bufs=2))
        yfp = ctx.enter_context(tc.tile_pool(name="yfp", bufs=2))
        bcp = ctx.enter_context(tc.tile_pool(name="bcp", bufs=2))

        # prologue-scoped pools (space reclaimed before the attention pools open)
        pro = ExitStack()
        xpool = pro.enter_context(tc.tile_pool(name="xpool", bufs=1))
        stats = pro.enter_context(tc.tile_pool(name="stats", bufs=1))
        stats2 = pro.enter_context(tc.tile_pool(name="stats2", bufs=2))
        v8p = pro.enter_context(tc.tile_pool(name="v8p", bufs=2))
        ppt = pro.enter_context(tc.tile_pool(name="ppt", bufs=2, space="PSUM"))
        ppsm = pro.enter_context(tc.tile_pool(name="ppsm", bufs=2, space="PSUM"))
        ppj = pro.enter_context(tc.tile_pool(name="ppj", bufs=2, space="PSUM"))

        # ---- constants / identities ----
        ident = consts.tile([P, P], f32)
        make_identity(nc, ident)
        identb = consts.tile([P, P], bf16)
        nc.vector.tensor_copy(out=identb, in_=ident)
        onesf = consts.tile([P, 2 * P], f32)
        nc.vector.memset(onesf, 1.0)
        ones8 = consts.tile([P, 2, P], f8)
        nc.vector.tensor_copy(out=ones8, in_=onesf[:].rearrange("p (u m) -> p u m", u=2))
        eps4 = consts.tile([NCH, 1], f32)
        nc.vector.memset(eps4, EPS)
        zero1 = consts.tile([P, 1], f32)
        nc.vector.memset(zero1, 0.0)
        # GN-folded fp8 projection weights: wX_s[:, c, :] = fp8(wX[:, c, :] * A_c)
        wq_s = consts.tile([P, NCH, HC], f8)
        wk_s = consts.tile([P, NCH, HC], f8)
        wv_s = consts.tile([P, NCH, HC], f8)

        # ---- load x (critical path): 4 slices per chunk so stats start early ----
        x8 = xpool.tile([P, NCH, N], f8)
        NSL = N // 4
        for ci in range(NCH):
            for sl in range(4):
                nc.sync.dma_start(
                    out=x8[:, ci, sl * NSL : (sl + 1) * NSL],
                    in_=x8d[:, ci, sl * NSL : (sl + 1) * NSL],
                )

        w_q = consts.tile([P, NCH, HC], f32)
        nc.sync.dma_start(out=w_q, in_=wqt)
        w_k = consts.tile([P, NCH, HC], f32)
        nc.sync.dma_start(out=w_k, in_=wkt)
        w_v = consts.tile([P, NCH, HC], f32)
        nc.sync.dma_start(out=w_v, in_=wvt)
        w_o = consts.tile([P, C], f32r)
        nc.sync.dma_start(out=w_o, in_=wot)
        bq2_h = consts.tile([64, 2], f32)
        nc.sync.dma_start(out=bq2_h, in_=bq2h)
        bv_sb = consts.tile([P, 1], f32)
        nc.sync.dma_start(out=bv_sb, in_=bvh)
        gns_h = [consts.tile([2, P], f32, name=f"gns{h}", tag=f"gns{h}") for h in range(2)]
        gnb_h = [consts.tile([2, P], f32, name=f"gnb{h}", tag=f"gnb{h}") for h in range(2)]
        gnsv = gns.rearrange("a (b c) -> (a b) c", b=NCH)
        gnbv = gnb.rearrange("a (b c) -> (a b) c", b=NCH)
        for h in range(2):
            nc.sync.dma_start(out=gns_h[h], in_=gnsv[2 * h : 2 * h + 2, :])
            nc.sync.dma_start(out=gnb_h[h], in_=gnbv[2 * h : 2 * h + 2, :])

        # ---- GroupNorm stats (from the fp8 x) per chunk-pair ----
        mv = stats.tile([P, NCH, 2], f32)
        acol = stats.tile([P, NCH], f32)
        bcol = stats.tile([P, NCH], f32)

        def gn_half(h):
            lo = 2 * h
            for ci in (lo, lo + 1):
                st = stats2.tile([P, 8, 6], f32, name="st", tag="st")
                xv = x8[:, ci, :].rearrange("p (s f) -> p s f", f=512)
                for s in range(8):
                    nc.vector.bn_stats(out=st[:, s, :], in_=xv[:, s, :])
                nc.vector.bn_aggr(out=mv[:, ci, :], in_=st)
            # vpm = var + mean^2
            vpm = stats.tile([P, 2], f32, name=f"vpm{h}", tag=f"vpm{h}")
            nc.vector.tensor_mul(vpm, mv[:, lo : lo + 2, 0], mv[:, lo : lo + 2, 0])
            nc.vector.tensor_add(vpm, vpm, mv[:, lo : lo + 2, 1])
            mrow = stats.tile([2, P], f32, name=f"mrow{h}", tag=f"mrow{h}")
            vrow = stats.tile([2, P], f32, name=f"vrow{h}", tag=f"vrow{h}")
            pmz = ppsm.tile([2, P], f32, name="pmz", tag="sm")
            nc.tensor.transpose(pmz, mv[:, lo : lo + 2, 0], ident)
            nc.vector.tensor_copy(out=mrow, in_=pmz)
            pvz = ppsm.tile([2, P], f32, name="pvz", tag="sm")
            nc.tensor.transpose(pvz, vpm, ident)
            nc.vector.tensor_copy(out=vrow, in_=pvz)
            gm = stats.tile([2, 8], f32, name=f"gm{h}", tag=f"gm{h}")
            gv = stats.tile([2, 8], f32, name=f"gv{h}", tag=f"gv{h}")
            nc.vector.reduce_sum(
                out=gm[:], in_=mrow[:].rearrange("p (g s) -> p g s", s=GSIZE), axis=AX.X
            )
            nc.vector.tensor_scalar_mul(gm, gm, 1.0 / GSIZE)
            nc.vector.reduce_sum(
                out=gv[:], in_=vrow[:].rearrange("p (g s) -> p g s", s=GSIZE), axis=AX.X
            )
            nc.vector.tensor_scalar_mul(gv, gv, 1.0 / GSIZE)
            gmsq = stats.tile([2, 8], f32, name=f"gmsq{h}", tag=f"gmsq{h}")
            nc.vector.tensor_mul(gmsq, gm, gm)
            nc.vector.tensor_sub(gv, gv, gmsq)     # group variance
            nc.scalar.activation(out=gv, in_=gv, func=AF.Sqrt, bias=eps4[0:2, :])
            nc.vector.reciprocal(gv, gv)           # rstd per group
            grx = stats.tile([2, P], f32, name=f"grx{h}", tag=f"grx{h}")
            gmx = stats.tile([2, P], f32, name=f"gmx{h}", tag=f"gmx{h}")
            gv_ap = gv[:]
            gm_ap = gm[:]
            gv_b = bass.AP(tensor=gv_ap.tensor, offset=gv_ap.offset, ap=list(gv_ap.ap) + [[0, GSIZE]])
            gm_b = bass.AP(tensor=gm_ap.tensor, offset=gm_ap.offset, ap=list(gm_ap.ap) + [[0, GSIZE]])
            nc.vector.tensor_copy(out=grx[:].rearrange("p (g s) -> p g s", s=GSIZE), in_=gv_b)
            nc.vector.tensor_copy(out=gmx[:].rearrange("p (g s) -> p g s", s=GSIZE), in_=gm_b)
            nc.vector.tensor_mul(grx, grx, gns_h[h])
            nc.vector.tensor_mul(gmx, gmx, grx)
            nc.vector.tensor_sub(gmx, gnb_h[h], gmx)
            paz = ppsm.tile([P, 2], f32, name="paz", tag="sm")
            nc.tensor.transpose(paz, grx, ident[0:2, 0:2])
            nc.vector.tensor_copy(out=acol[:, lo : lo + 2], in_=paz)
            pbz = ppsm.tile([P, 2], f32, name="pbz", tag="sm")
            nc.tensor.transpose(pbz, gmx, ident[0:2, 0:2])
            nc.vector.tensor_copy(out=bcol[:, lo : lo + 2], in_=pbz)
            # fold GN scale into the fp8 projection weights; x stays raw
            for ci in (lo, lo + 1):
                for wsrc, wdst in ((w_q, wq_s), (w_k, wk_s), (w_v, wv_s)):
                    nc.scalar.activation(
                        out=wdst[:, ci, :],
                        in_=wsrc[:, ci, :],
                        func=AF.Identity,
                        bias=zero1,
                        scale=acol[:, ci : ci + 1],
                    )

        gn_half(0)
        gn_half(1)

        # ---- bias folds: bvec = W^T B (per output channel) + conv bias ----
        # q: directly in [64, 2] split layout.  k: none needed (softmax-invariant).
        pbq2 = ppsm.tile([64, 2], f32, name="pbq2", tag="sm")
        for hh in range(2):
            for ci in range(NCH):
                nc.tensor.matmul(
                    pbq2[:, hh : hh + 1],
                    lhsT=w_q[:, ci, 64 * hh : 64 * hh + 64],
                    rhs=bcol[:, ci : ci + 1],
                    start=(ci == 0),
                    stop=(ci == NCH - 1),
                )
        bq2 = stats.tile([64, 2], f32, name="bq2", tag="bq2")
        nc.vector.tensor_add(bq2, bq2_h, pbq2)
        pbv = ppsm.tile([P, 1], f32, name="pbv", tag="sm")
        for ci in range(NCH):
            nc.tensor.matmul(
                pbv,
                lhsT=w_v[:, ci, :],
                rhs=bcol[:, ci : ci + 1],
                start=(ci == 0),
                stop=(ci == NCH - 1),
            )
        b2v = stats.tile([P, 1], f32, name="b2v", tag="b2v")
        nc.vector.tensor_add(b2v, bv_sb, pbv)

        # ---- projections (fp8 DoubleRow over input-chunk pairs) ----
        # k2/q2: split layout [64, half, N]; vt: [128j, jt, c] fp8
        k2 = qkv.tile([64, 2, N], f8)
        q2 = qkv.tile([64, 2, N], f8)
        vt = qkv.tile([P, NJT, HC], f8)

        for g in range(NIG):
            gs = slice(g * IG, (g + 1) * IG)
            psk = ppj.tile([64, 2, IG], f32, tag="pj")
            for hh in range(2):
                for t in range(2):
                    nc.tensor.matmul(
                        psk[:, hh, :],
                        lhsT=wk_s[:, 2 * t : 2 * t + 2, 64 * hh : 64 * hh + 64],
                        rhs=x8[:, 2 * t : 2 * t + 2, gs],
                        start=(t == 0),
                        stop=(t == 1),
                        perf_mode=DR,
                    )
            nc.vector.tensor_copy(out=k2[:, :, gs], in_=psk)

        for g in range(NIG):
            gs = slice(g * IG, (g + 1) * IG)
            psv = ppj.tile([P, IG], f32, tag="pj")
            for t in range(2):
                nc.tensor.matmul(
                    psv,
                    lhsT=wv_s[:, 2 * t : 2 * t + 2, :],
                    rhs=x8[:, 2 * t : 2 * t + 2, gs],
                    start=(t == 0),
                    stop=(t == 1),
                    perf_mode=DR,
                )
            v8 = v8p.tile([P, IG], bf16, tag="v8")
            nc.vector.tensor_scalar_add(v8, psv, b2v)
            for j in range(IG // P):
                jt = g * (IG // P) + j
                ptr = ppt.tile([P, P], bf16)
                nc.tensor.transpose(ptr, v8[:, j * P : (j + 1) * P], identb)
                nc.vector.tensor_copy(out=vt[:, jt, :], in_=ptr)

        for g in range(NIG):
            gs = slice(g * IG, (g + 1) * IG)
            psq = ppj.tile([64, 2, IG], f32, tag="pj")
            for hh in range(2):
                for t in range(2):
                    nc.tensor.matmul(
                        psq[:, hh, :],
                        lhsT=wq_s[:, 2 * t : 2 * t + 2, 64 * hh : 64 * hh + 64],
                        rhs=x8[:, 2 * t : 2 * t + 2, gs],
                        start=(t == 0),
                        stop=(t == 1),
                        perf_mode=DR,
                    )
            for hh in range(2):
                nc.vector.tensor_scalar_add(q2[:, hh, gs], psq[:, hh, :], bq2[:, hh : hh + 1])

        pro.close()

        # attention-phase pools
        ptp = ctx.enter_context(tc.tile_pool(name="ptp", bufs=2))
        pps = ctx.enter_context(tc.tile_pool(name="pps", bufs=2, space="PSUM"))
        ppden = ctx.enter_context(tc.tile_pool(name="ppden", bufs=1, space="PSUM"))
        ppo = ctx.enter_context(tc.tile_pool(name="ppo", bufs=1, space="PSUM"))
        ppf = ctx.enter_context(tc.tile_pool(name="ppf", bufs=2, space="PSUM"))

        # ---- attention ----
        for g in range(NIG):
            gs = slice(g * IG, (g + 1) * IG)
            qg = q2[:, :, gs]
            pt8 = ptp.tile([P, NJT, IG], f8, tag="pt8")

            # S^T pair matmuls (DoubleRow, contraction = 2 x 64 channels)
            # + one exp per 1024 columns straight to fp8
            for u in range(NJT // 2):
                ps = pps.tile([P, 2, IG], f32, tag="ps")
                for h in range(2):
                    jt = 2 * u + h
                    nc.tensor.matmul(
                        ps[:, h, :],
                        lhsT=k2[:, :, jt * P : (jt + 1) * P],
                        rhs=qg,
                        start=True,
                        stop=True,
                        perf_mode=DR,
                    )
                nc.scalar.activation(
                    out=pt8[:, 2 * u : 2 * u + 2, :],
                    in_=ps,
                    func=AF.Exp,
                    scale=SCALE,
                )

            # denominators: all-ones DoubleRow matmuls -> [128, 512] (rows equal)
            pden = ppden.tile([P, IG], f32)
            for u in range(NJT // 2):
                nc.tensor.matmul(
                    pden,
                    lhsT=ones8,
                    rhs=pt8[:, 2 * u : 2 * u + 2, :],
                    start=(u == 0),
                    stop=(u == NJT // 2 - 1),
                    perf_mode=DR,
                )

            po = ppo.tile([P, IG], f32)
            for u in range(NJT // 2):
                nc.tensor.matmul(
                    po,
                    lhsT=vt[:, 2 * u : 2 * u + 2, :],
                    rhs=pt8[:, 2 * u : 2 * u + 2, :],
                    start=(u == 0),
                    stop=(u == NJT // 2 - 1),
                    perf_mode=DR,
                )

            bc = bcp.tile([P, IG], f32)
            nc.vector.reciprocal_approx_fast(out=bc, in_=pden)
            ot = otp.tile([P, IG], f32r)
            nc.vector.tensor_mul(ot, po, bc)

            for oc in range(NCH):
                pf = ppf.tile([P, IG], f32)
                nc.tensor.matmul(pf, lhsT=w_o[:, oc * P : (oc + 1) * P], rhs=ot, start=True, stop=True)
                yf = yfp.tile([P, IG], bf16)
                nc.vector.tensor_copy(out=yf, in_=pf)
                nc.sync.dma_start(out=ypv[oc, :, g, :], in_=yf)

    nc.compile()
    return nc


def get_nc():
    if "nc" not in _NC_CACHE:
        _NC_CACHE["nc"] = _build_nc()
    return _NC_CACHE["nc"]


def make_in_maps(inputs):
    f8 = ml_dtypes.float8_e4m3
    x = np.asarray(inputs["x"], np.float32)
    wq = np.asarray(inputs["wq"], np.float32)
    wk = np.asarray(inputs["wk"], np.float32)
    wv = np.asarray(inputs["wv"], np.float32)
    bq = np.asarray(inputs["bq"], np.float32)
    bv = np.asarray(inputs["bv"], np.float32)
    wo = np.asarray(inputs["wo"], np.float32)
    gn_scale = np.asarray(inputs["gn_scale"], np.float32)
    gn_bias = np.asarray(inputs["gn_bias"], np.float32)

    # x8[b]: [128p, 4chunk, 4096] fp8, channel c = chunk*128 + p
    x8s = [
        np.ascontiguousarray(
            x[b].reshape(NCH, P, N).transpose(1, 0, 2).astype(f8)
        )
        for b in range(2)
    ]

    def wt(w, sl):
        # [128 p_in, 4 chunk, 128 out]: wt[p, a, o] = w[sl][o, a*128+p]
        return np.ascontiguousarray(w[sl, :].T.reshape(NCH, P, HC).transpose(1, 0, 2))

    in_maps = []
    for cid in range(8):
        b, h = divmod(cid, HEADS)
        sl = slice(h * HC, (h + 1) * HC)
        in_maps.append(
            {
                "x8d": x8s[b],
                "wqt": wt(wq, sl),
                "wkt": wt(wk, sl),
                "wvt": wt(wv, sl),
                "wot": np.ascontiguousarray(wo[:, sl].T),
                "bq2h": np.ascontiguousarray(bq[sl].reshape(2, 64).T),
                "bvh": np.ascontiguousarray(bv[sl].reshape(HC, 1)),
                "gns": np.ascontiguousarray(gn_scale.reshape(1, C)),
                "gnb": np.ascontiguousarray(gn_bias.reshape(1, C)),
            }
        )
    return in_maps


def assemble_output(inputs, yps):
    x = np.asarray(inputs["x"], np.float32)
    bo = np.asarray(inputs["bo"], np.float32)
    y = x.reshape(2, C, N).astype(np.float32).copy()
    y += bo.reshape(1, C, 1)
    for cid in range(8):
        b = cid // HEADS
        y[b] += np.asarray(yps[cid]).astype(np.float32)
    return y.reshape(2, C, 64, 64)


def run(inputs, trace=False):
    from concourse.bass_utils import run_bass_kernel_spmd

    nc = get_nc()
    in_maps = make_in_maps(inputs)
    res = run_bass_kernel_spmd(nc, in_maps, list(range(8)), trace=trace)
    yps = [r["yp"] for r in res.results]
    return assemble_output(inputs, yps), res


def kernel(**inputs):
    y, _ = run(inputs, trace=False)
    return y


# revision 5
# speedup vs baseline: 1.3143x; 1.2089x over previous
"""Trainium2 Bass kernel for nn_AttnBlock (GroupNorm + 4-head attention + output proj).

Sharding: 8 cores = (batch b in {0,1}) x (head h in {0..3}).  Each core computes
the full attention for its (b, h) pair plus the partial output projection
wo[:, head_cols] @ att_out_head -> [512, 4096] (bf16).  The host sums the 4
head partials per batch and adds the residual x and output bias bo.

v3 (fp8 everywhere on the attention path, measured-HW-model driven):
  - x arrives fp8 [128p, 4chunk, 4096], one big DMA per chunk (4KB contiguous
    per partition -> ~300GB/s).  GroupNorm stats (bn_stats) read fp8 directly;
    GN is folded into fp8 projection weights + biases; x is never rewritten.
  - q,k,v projections: fp8 DoubleRow over input-chunk pairs ([128,2,128] lhsT
    x [128,2,512] rhs), two accumulating matmuls per 512-query group.
    k's bias is dropped (a per-query-constant score shift cancels in softmax).
  - S^T[j,i] per key-tile: fp8 DoublePixel matmul (2 moving cols/cycle).
  - exp on ACT (the wall: 128 x ~1.1us), fp8 out into double-buffered pt8.
  - denominator via all-ones DoubleRow matmuls; out^T = V P DoubleRow.
  - wo projection: fp8 DoublePixel; y written bf16 (host upcasts, adds
    residual + bo).
  - DVE: bn_stats, psum->sbuf conversions, reciprocal_approx_fast, ot=po/den.
"""

import sys

sys.path.insert(0, "/opt/trn_rl_repo")

import numpy as np
import ml_dtypes

C = 512
HEADS = 4
HC = 128          # head channels
N = 4096          # h*w pixels
P = 128           # partitions
NCH = C // P      # 4 channel chunks
NJT = N // P      # 32 key tiles
IG = 512          # query-group width
NIG = N // IG     # 8 query groups
GSIZE = 16        # channels per groupnorm group
EPS = 1e-6
SCALE = float(C) ** -0.5

_NC_CACHE = {}


def _build_nc():
    from contextlib import ExitStack

    import concourse.bacc as bacc
    import concourse.bass as bass
    import concourse.tile as tile
    from concourse import mybir
    from concourse.masks import make_identity

    f32 = mybir.dt.float32
    bf16 = mybir.dt.bfloat16
    f8 = mybir.dt.float8e4

    AF = mybir.ActivationFunctionType
    AX = mybir.AxisListType
    DR = mybir.MatmulPerfMode.DoubleRow
    DP = mybir.MatmulPerfMode.DoublePixel

    nc = bacc.Bacc("TRN2", target_bir_lowering=False, debug=False)

    x8d = nc.dram_tensor("x8d", [P, NCH, N], f8, kind="ExternalInput").ap()
    wqt = nc.dram_tensor("wqt", [P, NCH, HC], f32, kind="ExternalInput").ap()
    wkt = nc.dram_tensor("wkt", [P, NCH, HC], f32, kind="ExternalInput").ap()
    wvt = nc.dram_tensor("wvt", [P, NCH, HC], f32, kind="ExternalInput").ap()
    wo8 = nc.dram_tensor("wo8", [HC, C], f8, kind="ExternalInput").ap()
    bqh = nc.dram_tensor("bqh", [HC, 1], f32, kind="ExternalInput").ap()
    bvh = nc.dram_tensor("bvh", [HC, 1], f32, kind="ExternalInput").ap()
    gns = nc.dram_tensor("gns", [1, C], f32, kind="ExternalInput").ap()
    gnb = nc.dram_tensor("gnb", [1, C], f32, kind="ExternalInput").ap()
    yp = nc.dram_tensor("yp", [C, N], bf16, kind="ExternalOutput").ap()

    ypv = yp.rearrange("(oc p) (g i) -> oc p g i", p=P, i=IG)  # [4, 128, 8, 512]

    with tile.TileContext(nc) as tc, ExitStack() as ctx:
        consts = ctx.enter_context(tc.tile_pool(name="consts", bufs=1))
        qkv = ctx.enter_context(tc.tile_pool(name="qkv", bufs=1))
        otp = ctx.enter_context(tc.tile_pool(name="otp", bufs=2))
        yfp = ctx.enter_context(tc.tile_pool(name="yfp", bufs=2))
        bcp = ctx.enter_context(tc.tile_pool(name="bcp", bufs=2))

        # prologue-scoped pools (space reclaimed before the attention pools open)
        pro = ExitStack()
        xpool = pro.enter_context(tc.tile_pool(name="xpool", bufs=1))
        stats = pro.enter_context(tc.tile_pool(name="stats", bufs=1))
        stats2 = pro.enter_context(tc.tile_pool(name="stats2", bufs=2))
        v8p = pro.enter_context(tc.tile_pool(name="v8p", bufs=2))
        ppt = pro.enter_context(tc.tile_pool(name="ppt", bufs=2, space="PSUM"))
        ppsm = pro.enter_context(tc.tile_pool(name="ppsm", bufs=2, space="PSUM"))
        ppj = pro.enter_context(tc.tile_pool(name="ppj", bufs=2, space="PSUM"))

        # ---- constants / identities ----
        ident = consts.tile([P, P], f32)
        make_identity(nc, ident)
        identb = consts.tile([P, P], bf16)
        nc.vector.tensor_copy(out=identb, in_=ident)
        onesf = consts.tile([P, 2 * P], f32)
        nc.vector.memset(onesf, 1.0)
        ones8 = consts.tile([P, 2, P], f8)
        nc.vector.tensor_copy(out=ones8, in_=onesf[:].rearrange("p (u m) -> p u m", u=2))
        eps4 = consts.tile([NCH, 1], f32)
        nc.vector.memset(eps4, EPS)
        zero1 = consts.tile([P, 1], f32)
        nc.vector.memset(zero1, 0.0)
        # GN-folded fp8 projection weights: wX_s[:, c, :] = fp8(wX[:, c, :] * A_c)
        wq_s = consts.tile([P, NCH, HC], f8)
        wk_s = consts.tile([P, NCH, HC], f8)
        wv_s = consts.tile([P, NCH, HC], f8)

        # ---- load x: one big DMA per chunk (4KB contiguous per partition) ----
        x8 = xpool.tile([P, NCH, N], f8)
        for ci in range(NCH):
            nc.sync.dma_start(out=x8[:, ci, :], in_=x8d[:, ci, :])

        w_q = consts.tile([P, NCH, HC], f32)
        nc.sync.dma_start(out=w_q, in_=wqt)
        w_k = consts.tile([P, NCH, HC], f32)
        nc.sync.dma_start(out=w_k, in_=wkt)
        w_v = consts.tile([P, NCH, HC], f32)
        nc.sync.dma_start(out=w_v, in_=wvt)
        w_o = consts.tile([P, C], f8)
        nc.sync.dma_start(out=w_o, in_=wo8)
        bq_sb = consts.tile([P, 1], f32)
        nc.sync.dma_start(out=bq_sb, in_=bqh)
        bv_sb = consts.tile([P, 1], f32)
        nc.sync.dma_start(out=bv_sb, in_=bvh)
        gns_h = [consts.tile([2, P], f32, name=f"gns{h}", tag=f"gns{h}") for h in range(2)]
        gnb_h = [consts.tile([2, P], f32, name=f"gnb{h}", tag=f"gnb{h}") for h in range(2)]
        gnsv = gns.rearrange("a (b c) -> (a b) c", b=NCH)
        gnbv = gnb.rearrange("a (b c) -> (a b) c", b=NCH)
        for h in range(2):
            nc.sync.dma_start(out=gns_h[h], in_=gnsv[2 * h : 2 * h + 2, :])
            nc.sync.dma_start(out=gnb_h[h], in_=gnbv[2 * h : 2 * h + 2, :])

        # ---- GroupNorm stats (from the fp8 x) per chunk-pair ----
        mv = stats.tile([P, NCH, 2], f32)
        acol = stats.tile([P, NCH], f32)
        bcol = stats.tile([P, NCH], f32)

        def gn_half(h):
            lo = 2 * h
            for ci in (lo, lo + 1):
                st = stats2.tile([P, 8, 6], f32, name="st", tag="st")
                xv = x8[:, ci, :].rearrange("p (s f) -> p s f", f=512)
                for s in range(8):
                    nc.vector.bn_stats(out=st[:, s, :], in_=xv[:, s, :])
                nc.vector.bn_aggr(out=mv[:, ci, :], in_=st)
            # vpm = var + mean^2
            vpm = stats.tile([P, 2], f32, name=f"vpm{h}", tag=f"vpm{h}")
            nc.vector.tensor_mul(vpm, mv[:, lo : lo + 2, 0], mv[:, lo : lo + 2, 0])
            nc.vector.tensor_add(vpm, vpm, mv[:, lo : lo + 2, 1])
            mrow = stats.tile([2, P], f32, name=f"mrow{h}", tag=f"mrow{h}")
            vrow = stats.tile([2, P], f32, name=f"vrow{h}", tag=f"vrow{h}")
            pmz = ppsm.tile([2, P], f32, name="pmz", tag="sm")
            nc.tensor.transpose(pmz, mv[:, lo : lo + 2, 0], ident)
            nc.vector.tensor_copy(out=mrow, in_=pmz)
            pvz = ppsm.tile([2, P], f32, name="pvz", tag="sm")
            nc.tensor.transpose(pvz, vpm, ident)
            nc.vector.tensor_copy(out=vrow, in_=pvz)
            gm = stats.tile([2, 8], f32, name=f"gm{h}", tag=f"gm{h}")
            gv = stats.tile([2, 8], f32, name=f"gv{h}", tag=f"gv{h}")
            nc.vector.reduce_sum(
                out=gm[:], in_=mrow[:].rearrange("p (g s) -> p g s", s=GSIZE), axis=AX.X
            )
            nc.vector.tensor_scalar_mul(gm, gm, 1.0 / GSIZE)
            nc.vector.reduce_sum(
                out=gv[:], in_=vrow[:].rearrange("p (g s) -> p g s", s=GSIZE), axis=AX.X
            )
            nc.vector.tensor_scalar_mul(gv, gv, 1.0 / GSIZE)
            gmsq = stats.tile([2, 8], f32, name=f"gmsq{h}", tag=f"gmsq{h}")
            nc.vector.tensor_mul(gmsq, gm, gm)
            nc.vector.tensor_sub(gv, gv, gmsq)     # group variance
            nc.scalar.activation(out=gv, in_=gv, func=AF.Sqrt, bias=eps4[0:2, :])
            nc.vector.reciprocal(gv, gv)           # rstd per group
            grx = stats.tile([2, P], f32, name=f"grx{h}", tag=f"grx{h}")
            gmx = stats.tile([2, P], f32, name=f"gmx{h}", tag=f"gmx{h}")
            gv_ap = gv[:]
            gm_ap = gm[:]
            gv_b = bass.AP(tensor=gv_ap.tensor, offset=gv_ap.offset, ap=list(gv_ap.ap) + [[0, GSIZE]])
            gm_b = bass.AP(tensor=gm_ap.tensor, offset=gm_ap.offset, ap=list(gm_ap.ap) + [[0, GSIZE]])
            nc.vector.tensor_copy(out=grx[:].rearrange("p (g s) -> p g s", s=GSIZE), in_=gv_b)
            nc.vector.tensor_copy(out=gmx[:].rearrange("p (g s) -> p g s", s=GSIZE), in_=gm_b)
            nc.vector.tensor_mul(grx, grx, gns_h[h])
            nc.vector.tensor_mul(gmx, gmx, grx)
            nc.vector.tensor_sub(gmx, gnb_h[h], gmx)
            paz = ppsm.tile([P, 2], f32, name="paz", tag="sm")
            nc.tensor.transpose(paz, grx, ident[0:2, 0:2])
            nc.vector.tensor_copy(out=acol[:, lo : lo + 2], in_=paz)
            pbz = ppsm.tile([P, 2], f32, name="pbz", tag="sm")
            nc.tensor.transpose(pbz, gmx, ident[0:2, 0:2])
            nc.vector.tensor_copy(out=bcol[:, lo : lo + 2], in_=pbz)
            # fold GN scale into the fp8 projection weights; x stays raw
            for ci in (lo, lo + 1):
                for wsrc, wdst in ((w_k, wk_s), (w_q, wq_s), (w_v, wv_s)):
                    nc.scalar.activation(
                        out=wdst[:, ci, :],
                        in_=wsrc[:, ci, :],
                        func=AF.Identity,
                        bias=zero1,
                        scale=acol[:, ci : ci + 1],
                    )

        gn_half(0)
        gn_half(1)

        # ---- bias folds: bvec = W^T B + conv bias (q and v only; k cancels) ----
        pbq = ppsm.tile([P, 1], f32, name="pbq", tag="sm")
        for ci in range(NCH):
            nc.tensor.matmul(
                pbq,
                lhsT=w_q[:, ci, :],
                rhs=bcol[:, ci : ci + 1],
                start=(ci == 0),
                stop=(ci == NCH - 1),
            )
        b2q = stats.tile([P, 1], f32, name="b2q", tag="b2q")
        nc.vector.tensor_add(b2q, bq_sb, pbq)
        pbv = ppsm.tile([P, 1], f32, name="pbv", tag="sm")
        for ci in range(NCH):
            nc.tensor.matmul(
                pbv,
                lhsT=w_v[:, ci, :],
                rhs=bcol[:, ci : ci + 1],
                start=(ci == 0),
                stop=(ci == NCH - 1),
            )
        b2v = stats.tile([P, 1], f32, name="b2v", tag="b2v")
        nc.vector.tensor_add(b2v, bv_sb, pbv)

        # ---- projections (fp8 DoubleRow over input-chunk pairs) ----
        k8 = qkv.tile([P, N], f8)
        q8 = qkv.tile([P, N], f8)
        vt = qkv.tile([P, NJT, HC], f8)

        def proj_group(g, w_s, out_fn):
            gs = slice(g * IG, (g + 1) * IG)
            ps = ppj.tile([P, IG], f32, tag="pj")
            for t in range(2):
                nc.tensor.matmul(
                    ps,
                    lhsT=w_s[:, 2 * t : 2 * t + 2, :],
                    rhs=x8[:, 2 * t : 2 * t + 2, gs],
                    start=(t == 0),
                    stop=(t == 1),
                    perf_mode=DR,
                )
            out_fn(ps, gs, g)

        def k_out(ps, gs, g):
            nc.vector.tensor_copy(out=k8[:, gs], in_=ps)

        def q_out(ps, gs, g):
            nc.vector.tensor_scalar_add(q8[:, gs], ps, b2q)

        def v_out(ps, gs, g):
            v8 = v8p.tile([P, IG], bf16, tag="v8")
            nc.vector.tensor_scalar_add(v8, ps, b2v)
            for j in range(IG // P):
                jt = g * (IG // P) + j
                ptr = ppt.tile([P, P], bf16)
                nc.tensor.transpose(ptr, v8[:, j * P : (j + 1) * P], identb)
                nc.vector.tensor_copy(out=vt[:, jt, :], in_=ptr)

        for g in range(NIG):
            proj_group(g, wk_s, k_out)
        proj_group(0, wq_s, q_out)
        for g in range(NIG):
            proj_group(g, wv_s, v_out)
        for g in range(1, NIG):
            proj_group(g, wq_s, q_out)

        pro.close()

        # attention-phase pools
        ptp = ctx.enter_context(tc.tile_pool(name="ptp", bufs=2))
        pps = ctx.enter_context(tc.tile_pool(name="pps", bufs=2, space="PSUM"))
        ppden = ctx.enter_context(tc.tile_pool(name="ppden", bufs=1, space="PSUM"))
        ppo = ctx.enter_context(tc.tile_pool(name="ppo", bufs=1, space="PSUM"))
        ppf = ctx.enter_context(tc.tile_pool(name="ppf", bufs=2, space="PSUM"))

        # ---- attention ----
        for g in range(NIG):
            gs = slice(g * IG, (g + 1) * IG)
            qg = q8[:, gs]
            pt8 = ptp.tile([P, NJT, IG], f8, tag="pt8")

            # S^T per key tile: fp8 DoublePixel matmuls; exp per 2 tiles -> fp8
            for u in range(NJT // 2):
                ps = pps.tile([P, 2, IG], f32, tag="ps")
                for h in range(2):
                    jt = 2 * u + h
                    nc.tensor.matmul(
                        ps[:, h, :],
                        lhsT=k8[:, jt * P : (jt + 1) * P],
                        rhs=qg,
                        start=True,
                        stop=True,
                        perf_mode=DP,
                    )
                nc.scalar.activation(
                    out=pt8[:, 2 * u : 2 * u + 2, :],
                    in_=ps,
                    func=AF.Exp,
                    scale=SCALE,
                )

            # denominators: all-ones DoubleRow matmuls -> [128, 512] (rows equal)
            pden = ppden.tile([P, IG], f32)
            for u in range(NJT // 2):
                nc.tensor.matmul(
                    pden,
                    lhsT=ones8,
                    rhs=pt8[:, 2 * u : 2 * u + 2, :],
                    start=(u == 0),
                    stop=(u == NJT // 2 - 1),
                    perf_mode=DR,
                )

            po = ppo.tile([P, IG], f32)
            for u in range(NJT // 2):
                nc.tensor.matmul(
                    po,
                    lhsT=vt[:, 2 * u : 2 * u + 2, :],
                    rhs=pt8[:, 2 * u : 2 * u + 2, :],
                    start=(u == 0),
                    stop=(u == NJT // 2 - 1),
                    perf_mode=DR,
                )

            bc = bcp.tile([P, IG], f32)
            nc.vector.reciprocal_approx_fast(out=bc, in_=pden)
            ot = otp.tile([P, IG], f8)
            nc.vector.tensor_mul(ot, po, bc)

            for oc in range(NCH):
                pf = ppf.tile([P, IG], f32)
                nc.tensor.matmul(
                    pf,
                    lhsT=w_o[:, oc * P : (oc + 1) * P],
                    rhs=ot,
                    start=True,
                    stop=True,
                    perf_mode=DP,
                )
                yf = yfp.tile([P, IG], bf16)
                nc.vector.tensor_copy(out=yf, in_=pf)
                nc.sync.dma_start(out=ypv[oc, :, g, :], in_=yf)

    nc.compile()
    return nc


def get_nc():
    if "nc" not in _NC_CACHE:
        _NC_CACHE["nc"] = _build_nc()
    return _NC_CACHE["nc"]


def make_in_maps(inputs):
    f8 = ml_dtypes.float8_e4m3
    x = np.asarray(inputs["x"], np.float32)
    wq = np.asarray(inputs["wq"], np.float32)
    wk = np.asarray(inputs["wk"], np.float32)
    wv = np.asarray(inputs["wv"], np.float32)
    bq = np.asarray(inputs["bq"], np.float32)
    bv = np.asarray(inputs["bv"], np.float32)
    wo = np.asarray(inputs["wo"], np.float32)
    gn_scale = np.asarray(inputs["gn_scale"], np.float32)
    gn_bias = np.asarray(inputs["gn_bias"], np.float32)

    # x8[b]: [128p, 4chunk, 4096] fp8, channel c = chunk*128 + p
    x8s = [
        np.ascontiguousarray(
            x[b].reshape(NCH, P, N).transpose(1, 0, 2).astype(f8)
        )
        for b in range(2)
    ]

    def wt(w, sl):
        # [128 p_in, 4 chunk, 128 out]: wt[p, a, o] = w[sl][o, a*128+p]
        return np.ascontiguousarray(w[sl, :].T.reshape(NCH, P, HC).transpose(1, 0, 2))

    in_maps = []
    for cid in range(8):
        b, h = divmod(cid, HEADS)
        sl = slice(h * HC, (h + 1) * HC)
        in_maps.append(
            {
                "x8d": x8s[b],
                "wqt": wt(wq, sl),
                "wkt": wt(wk, sl),
                "wvt": wt(wv, sl),
                "wo8": np.ascontiguousarray(wo[:, sl].T).astype(f8),
                "bqh": np.ascontiguousarray(bq[sl].reshape(HC, 1)),
                "bvh": np.ascontiguousarray(bv[sl].reshape(HC, 1)),
                "gns": np.ascontiguousarray(gn_scale.reshape(1, C)),
                "gnb": np.ascontiguousarray(gn_bias.reshape(1, C)),
            }
        )
    return in_maps


def assemble_output(inputs, yps):
    x = np.asarray(inputs["x"], np.float32)
    bo = np.asarray(inputs["bo"], np.float32)
    y = x.reshape(2, C, N).astype(np.float32).copy()
    y += bo.reshape(1, C, 1)
    for cid in range(8):
        b = cid // HEADS
        y[b] += np.asarray(yps[cid]).astype(np.float32)
    return y.reshape(2, C, 64, 64)


def run(inputs, trace=False):
    from concourse.bass_utils import run_bass_kernel_spmd

    nc = get_nc()
    in_maps = make_in_maps(inputs)
    res = run_bass_kernel_spmd(nc, in_maps, list(range(8)), trace=trace)
    yps = [r["yp"] for r in res.results]
    return assemble_output(inputs, yps), res


def kernel(**inputs):
    y, _ = run(inputs, trace=False)
    return y
